# revision 20
# baseline (speedup 1.0000x reference)
"""Trainium2 Bass kernel: 4096x4096 valid 5x5 cross-correlation + scalar bias.

Strategy (8 NeuronCores, SPMD):
  - Shard the OUTPUT by columns: core c computes out[:, 512c : 512c+512]
    (core 7's last 4 columns are padding, trimmed after gather). Each core
    reads x rows 0..4095, cols [512c, 512c+516) (host-padded to width 4100).
  - On-core: the 5x5 conv is computed as banded-matrix matmuls on the
    TensorEngine. For an input row-tile X_g = x[124g : 124g+128, :] and
    kernel column dj, the banded matrix B_dj[k, m] = w[k-m, dj] gives
      (B_dj^T @ X_g[:, dj:dj+512])[m, n] = sum_di w[di, dj] x[124g+m+di, n+dj]
    so accumulating the 5 dj-matmuls in PSUM yields 124 valid output rows
    per tile. 4092 = 33 * 124 exactly; rows 124g+p for p<128, g<33 cover
    0..4095 exactly (124*32+127 = 4095).
  - HOST-PACKED I/O LAYOUT: x is pre-gathered on the host into a
    [128, 33*516] bf16 array whose partition p, segment g holds
    x[124g+p, :].  The whole input then streams in a handful of dma_starts
    with multi-KB contiguous lines per partition (descriptor- and
    semaphore-overhead amortized; the DMA issue path costs ~0.65us + 900ns
    semaphore per dma_start, so 100+ small DMAs dominated the old
    timeline).  The output is staged to SBUF as [128, 33*512] fp32 in the
    same packed form and written out in chunk-sized dma_starts; the host
    un-packs (transpose) after the gather.
  - PSUM accumulation is fp32; bias fused into the PSUM->SBUF drain (DVE
    tensor_scalar_add).  Matmul operands are bf16 (accumulate fp32), which
    halves input HBM traffic; rel err ~2e-3.
  - WARM-UP: the Tensor engine ramps 0.65->1.2->2.4 GHz over ~7us of
    sustained activity.  A run of dummy matmuls on a memset tile starts at
    t~0.5us (during the input DMA lead-in) so the real matmul stream runs
    at full clock from its first instruction.
"""
import os

os.environ.setdefault("MYCRO_LOCAL_CACHE", "1")

import numpy as np

import concourse.bass as bass
import concourse.bacc as bacc
import concourse.tile as tile
import concourse.mybir as mybir
from concourse import bass_utils

H, W = 4096, 4096
KH, KW = 5, 5
OH, OW = H - KH + 1, W - KW + 1          # 4092, 4092
NCORES = 8
COLS = 512                               # output cols per core
XC = COLS + KW - 1                       # 516 input cols per core
NG = 33                                  # row tiles per core (33*124 = 4092)
RV = 124                                 # valid output rows per tile

# groups per input-chunk == per output-block. First chunk small so the
# first matmul starts early; last chunks small so the final drain+DMA tail
# is short. PSUM: two adjacent blocks in flight need <= 8 banks.
CHUNKS = [1, 2, 4, 4, 4, 4, 4, 4, 3, 2, 1]
assert sum(CHUNKS) == NG and max(a + b for a, b in zip(CHUNKS, CHUNKS[1:])) <= 8

# Dummy matmuls bridge the idle window between Tensor-sequencer-ready
# (~7.3us, after the framework preamble) and first-chunk-consumable
# (~9us), keeping the clock ramp (0.65->1.2->2.4 GHz over ~4.5us of
# CONTINUOUS activity) going — an idle gap resets the ramp. 256-row warm
# matmuls (~0.2us each) give fine granularity so warm-up ends close to
# data-ready; real matmuls do useful work while the ramp completes.
WARM_MM = 12
WARM_ROWS = 256
BT = KW * 128                             # banded-weight cols at xs[:, 0:BT]

_compiled = None
TRACE = False            # test harness can flip this for neuron-profile timing
LAST_EXEC_NS = None


def _build():
    nc = bacc.Bacc("TRN2", target_bir_lowering=False, debug=False,
                   num_devices=NCORES)
    mdt = mybir.dt.bfloat16

    # xs = [banded weights | packed x]: one dtype, so the first dma_start
    # delivers the weights AND the first row-group together (each push
    # costs ~0.65us descgen + 0.9us semaphore on the critical path).
    x_dram = nc.dram_tensor("xs", (128, BT + NG * XC), mdt,
                            kind="ExternalInput")
    bias_dram = nc.dram_tensor("biast", (128, 1), mybir.dt.float32,
                               kind="ExternalInput")
    # NOTE: full 128 partitions on purpose — a 124-row DMA falls off the
    # HWDGE fast path (DIRECT2D descgen takes ~8us per push vs ~0.3us).
    out_dram = nc.dram_tensor("out", (128, NG * COLS), mybir.dt.float32,
                              kind="ExternalOutput")

    with tile.TileContext(nc) as tc:
        with (
            tc.tile_pool(name="const", bufs=1) as cpool,
            tc.tile_pool(name="x", bufs=len(CHUNKS)) as xpool,
            tc.tile_pool(name="stage", bufs=len(CHUNKS)) as spool,
            tc.tile_pool(name="psum", bufs=8, space=bass.MemorySpace.PSUM) as ppool,
        ):
            # PE warm-up: memset on GpSimd (idle at kernel start, so the
            # producer dependency resolves as early as possible), then
            # dummy matmuls into a scratch PSUM bank that is never read.
            warm = cpool.tile([128, COLS], mdt)
            nc.gpsimd.memset(warm[:], 0.0)
            wps = ppool.tile([128, COLS], mybir.dt.float32, name="warmps",
                             tag="ps")
            for i in range(WARM_MM):
                nc.tensor.matmul(wps[:, 0:WARM_ROWS], warm[:, 0:128],
                                 warm[:, 0:WARM_ROWS],
                                 start=True, stop=True)

            biast = cpool.tile([128, 1], mybir.dt.float32)
            bt = cpool.tile([128, BT], mdt)
            # Parallel critical-path loads: weights via sync ring, group 0
            # via the (otherwise idle until the first drain) scalar ring —
            # the two ~0.7us descgens and transfers overlap instead of
            # serializing, so the first real matmul starts ~1.5us earlier.
            nc.sync.dma_start(bt[:], x_dram.ap()[:, 0:BT])
            xts, off = [], 0
            for k, ck in enumerate(CHUNKS):
                xt = xpool.tile([128, ck * XC], mdt, tag="x")
                # chunks 0 and 1 ride the scalar ring: their descgens run
                # in parallel with bt/chunk-2 descgen on sync, so both meet
                # their consume deadlines (~10.4us and ~11.0us) despite the
                # ~1.4us post-transfer semaphore latency per push.
                ring = nc.scalar if k <= 1 else nc.sync
                ring.dma_start(
                    xt[:],
                    x_dram.ap()[:, BT + off * XC:BT + (off + ck) * XC])
                xts.append(xt)
                off += ck
                if k == 1:
                    nc.scalar.dma_start(biast[:], bias_dram.ap())

            off = 0
            for k, ck in enumerate(CHUNKS):
                base = 0
                stg = spool.tile([128, ck * COLS], mybir.dt.float32)
                psts = []
                for gl in range(ck):
                    psts.append(ppool.tile([128, COLS], mybir.dt.float32,
                                           name=f"ps{off + gl}", tag="ps"))
                # weight-stationary sweep: dj outer, groups inner (pipelines
                # PSUM accumulation across banks)
                for dj in range(KW):
                    for gl in range(ck):
                        nc.tensor.matmul(
                            psts[gl][:],
                            bt[:, dj * 128:(dj + 1) * 128],
                            xts[k][:, base + gl * XC + dj:
                                   base + gl * XC + dj + COLS],
                            start=(dj == 0),
                            stop=(dj == KW - 1),
                        )
                # drain PSUM -> stage with fused bias, split across DVE and
                # ACT so a block's drains run in ~half the serial time
                for gl in range(ck):
                    dst = stg[:, gl * COLS:(gl + 1) * COLS]
                    if gl % 2 == 0:
                        nc.vector.tensor_scalar_add(dst, psts[gl][:],
                                                    biast[:])
                    else:
                        nc.scalar.activation(
                            dst, psts[gl][:],
                            mybir.ActivationFunctionType.Identity,
                            bias=biast[:])
                # one output DMA per block, scalar ring (keeps the sync
                # ring free for input); the last block goes on sync (idle
                # by then) so its descgen overlaps scalar's previous block.
                ring = nc.sync if k == len(CHUNKS) - 1 else nc.scalar
                ring.dma_start(
                    out_dram.ap()[:, off * COLS:(off + ck) * COLS], stg[:])
                off += ck

    nc.compile()
    return nc


def _banded(weight: np.ndarray) -> np.ndarray:
    ball = np.zeros((128, KW * 128), dtype=np.float32)
    for dj in range(KW):
        for di in range(KH):
            m = np.arange(128 - di)
            ball[m + di, dj * 128 + m] = weight[di, dj]
    return ball


def kernel(x: np.ndarray, weight: np.ndarray, bias: np.ndarray) -> np.ndarray:
    global _compiled
    import ml_dtypes
    bf16 = ml_dtypes.bfloat16

    x = np.asarray(x, dtype=np.float32)
    weight = np.asarray(weight, dtype=np.float32)
    bias = np.asarray(bias, dtype=np.float32)

    if _compiled is None:
        _compiled = _build()
    nc = _compiled

    xpad = np.zeros((H, NCORES * COLS + KW - 1), dtype=bf16)
    xpad[:, :W] = x.astype(bf16)
    ball = _banded(weight).astype(bf16)
    bias_col = np.full((128, 1), bias[0], dtype=np.float32)

    # pack: xs = [banded weights | xp], xp[p, g*XC + c] = x[124g+p, 512c0+c]
    idx = (124 * np.arange(NG)[:, None] + np.arange(128)[None, :])  # (NG,128)
    in_maps = []
    for c in range(NCORES):
        xc = xpad[:, COLS * c: COLS * c + XC]      # (4096, XC) view
        xp = xc[idx, :]                            # (NG, 128, XC)
        xs = np.empty((128, BT + NG * XC), dtype=bf16)
        xs[:, :BT] = ball
        xs[:, BT:] = xp.transpose(1, 0, 2).reshape(128, NG * XC)
        in_maps.append({"xs": xs, "biast": bias_col})

    res = bass_utils.run_bass_kernel_spmd(nc, in_maps,
                                          core_ids=list(range(NCORES)),
                                          trace=TRACE)
    global LAST_EXEC_NS
    LAST_EXEC_NS = res.exec_time_ns

    # unpack: out[124g + m, 512c + n] = op[m, g*COLS + n]  (m < 124)
    cols = []
    for c in range(NCORES):
        op = res.results[c]["out"].reshape(128, NG, COLS)
        cols.append(op[:RV].transpose(1, 0, 2).reshape(OH, COLS))
    out = np.hstack(cols)
    return np.ascontiguousarray(out[:, :OW])


# revision 21
# speedup vs baseline: 1.0402x; 1.0402x over previous
"""Trainium2 Bass kernel: 4096x4096 valid 5x5 cross-correlation + scalar bias.

Strategy (8 NeuronCores, SPMD):
  - Shard the OUTPUT by columns: core c computes out[:, 512c : 512c+512]
    (core 7's last 4 columns are padding, trimmed after gather). Each core
    reads x rows 0..4095, cols [512c, 512c+516) (host-padded to width 4100).
  - On-core: the 5x5 conv is computed as banded-matrix matmuls on the
    TensorEngine. For an input row-tile X_g = x[124g : 124g+128, :] and
    kernel column dj, the banded matrix B_dj[k, m] = w[k-m, dj] gives
      (B_dj^T @ X_g[:, dj:dj+512])[m, n] = sum_di w[di, dj] x[124g+m+di, n+dj]
    so accumulating the 5 dj-matmuls in PSUM yields 124 valid output rows
    per tile. 4092 = 33 * 124 exactly; rows 124g+p for p<128, g<33 cover
    0..4095 exactly (124*32+127 = 4095).
  - HOST-PACKED I/O LAYOUT: x is pre-gathered on the host into a
    [128, 33*516] bf16 array whose partition p, segment g holds
    x[124g+p, :].  The whole input then streams in a handful of dma_starts
    with multi-KB contiguous lines per partition (descriptor- and
    semaphore-overhead amortized; the DMA issue path costs ~0.65us + 900ns
    semaphore per dma_start, so 100+ small DMAs dominated the old
    timeline).  The output is staged to SBUF as [128, 33*512] fp32 in the
    same packed form and written out in chunk-sized dma_starts; the host
    un-packs (transpose) after the gather.
  - PSUM accumulation is fp32; bias fused into the PSUM->SBUF drain (DVE
    tensor_scalar_add).  Matmul operands are bf16 (accumulate fp32), which
    halves input HBM traffic; rel err ~2e-3.
  - WARM-UP: the Tensor engine ramps 0.65->1.2->2.4 GHz over ~7us of
    sustained activity.  A run of dummy matmuls on a memset tile starts at
    t~0.5us (during the input DMA lead-in) so the real matmul stream runs
    at full clock from its first instruction.
"""
import os

os.environ.setdefault("MYCRO_LOCAL_CACHE", "1")

import numpy as np

import concourse.bass as bass
import concourse.bacc as bacc
import concourse.tile as tile
import concourse.mybir as mybir
from concourse import bass_utils

H, W = 4096, 4096
KH, KW = 5, 5
OH, OW = H - KH + 1, W - KW + 1          # 4092, 4092
NCORES = 8
COLS = 512                               # output cols per core
XC = COLS + KW - 1                       # 516 input cols per core
NG = 33                                  # row tiles per core (33*124 = 4092)
RV = 124                                 # valid output rows per tile

# groups per input-chunk == per output-block. First chunk small so the
# first matmul starts early; last chunks small so the final drain+DMA tail
# is short. PSUM: two adjacent blocks in flight need <= 8 banks.
CHUNKS = [1, 2, 4, 4, 4, 4, 4, 4, 3, 2, 1]
assert sum(CHUNKS) == NG and max(a + b for a, b in zip(CHUNKS, CHUNKS[1:])) <= 8

# Dummy matmuls bridge the idle window between Tensor-sequencer-ready
# (~7.3us, after the framework preamble) and first-chunk-consumable
# (~9us), keeping the clock ramp (0.65->1.2->2.4 GHz over ~4.5us of
# CONTINUOUS activity) going — an idle gap resets the ramp. 256-row warm
# matmuls (~0.2us each) give fine granularity so warm-up ends close to
# data-ready; real matmuls do useful work while the ramp completes.
WARM_MM = 12
WARM_ROWS = 256
BT = KW * 128                             # banded-weight cols at xs[:, 0:BT]

_compiled = None
TRACE = False            # test harness can flip this for neuron-profile timing
LAST_EXEC_NS = None


def _build():
    nc = bacc.Bacc("TRN2", target_bir_lowering=False, debug=False,
                   num_devices=NCORES)
    mdt = mybir.dt.bfloat16

    # xs = [banded weights | packed x]: one dtype, so the first dma_start
    # delivers the weights AND the first row-group together (each push
    # costs ~0.65us descgen + 0.9us semaphore on the critical path).
    x_dram = nc.dram_tensor("xs", (128, BT + NG * XC), mdt,
                            kind="ExternalInput")
    bias_dram = nc.dram_tensor("biast", (128, 1), mybir.dt.float32,
                               kind="ExternalInput")
    # NOTE: full 128 partitions on purpose — a 124-row DMA falls off the
    # HWDGE fast path (DIRECT2D descgen takes ~8us per push vs ~0.3us).
    out_dram = nc.dram_tensor("out", (128, NG * COLS), mybir.dt.float32,
                              kind="ExternalOutput")

    with tile.TileContext(nc) as tc:
        with (
            tc.tile_pool(name="const", bufs=1) as cpool,
            tc.tile_pool(name="x", bufs=len(CHUNKS)) as xpool,
            tc.tile_pool(name="stage", bufs=len(CHUNKS)) as spool,
            tc.tile_pool(name="psum", bufs=8, space=bass.MemorySpace.PSUM) as ppool,
        ):
            # PE warm-up: memset on GpSimd (idle at kernel start, so the
            # producer dependency resolves as early as possible), then
            # dummy matmuls into a scratch PSUM bank that is never read.
            warm = cpool.tile([128, COLS], mdt)
            nc.gpsimd.memset(warm[:], 0.0)
            wps = ppool.tile([128, COLS], mybir.dt.float32, name="warmps",
                             tag="ps")
            for i in range(WARM_MM):
                nc.tensor.matmul(wps[:, 0:WARM_ROWS], warm[:, 0:128],
                                 warm[:, 0:WARM_ROWS],
                                 start=True, stop=True)

            biast = cpool.tile([128, 1], mybir.dt.float32)
            bt = cpool.tile([128, BT], mdt)
            # Parallel critical-path loads: weights via sync ring, group 0
            # via the (otherwise idle until the first drain) scalar ring —
            # the two ~0.7us descgens and transfers overlap instead of
            # serializing, so the first real matmul starts ~1.5us earlier.
            nc.sync.dma_start(bt[:], x_dram.ap()[:, 0:BT])
            xts, off = [], 0
            for k, ck in enumerate(CHUNKS):
                xt = xpool.tile([128, ck * XC], mdt, tag="x")
                # chunk 0 rides the scalar ring: its descgen runs in
                # parallel with bt's on sync, so the first matmul's two
                # inputs arrive together despite the ~0.7us/push descgen
                # and ~1.4us post-transfer semaphore latency.
                ring = nc.scalar if k == 0 else nc.sync
                ring.dma_start(
                    xt[:],
                    x_dram.ap()[:, BT + off * XC:BT + (off + ck) * XC])
                xts.append(xt)
                off += ck
                if k == 0:
                    nc.scalar.dma_start(biast[:], bias_dram.ap())

            off = 0
            for k, ck in enumerate(CHUNKS):
                base = 0
                stg = spool.tile([128, ck * COLS], mybir.dt.float32)
                psts = []
                for gl in range(ck):
                    psts.append(ppool.tile([128, COLS], mybir.dt.float32,
                                           name=f"ps{off + gl}", tag="ps"))
                # weight-stationary sweep: dj outer, groups inner (pipelines
                # PSUM accumulation across banks)
                for dj in range(KW):
                    for gl in range(ck):
                        nc.tensor.matmul(
                            psts[gl][:],
                            bt[:, dj * 128:(dj + 1) * 128],
                            xts[k][:, base + gl * XC + dj:
                                   base + gl * XC + dj + COLS],
                            start=(dj == 0),
                            stop=(dj == KW - 1),
                        )
                # drain PSUM -> stage with fused bias, split across DVE and
                # ACT so a block's drains run in ~half the serial time
                for gl in range(ck):
                    dst = stg[:, gl * COLS:(gl + 1) * COLS]
                    if gl % 2 == 0:
                        nc.vector.tensor_scalar_add(dst, psts[gl][:],
                                                    biast[:])
                    else:
                        nc.scalar.activation(
                            dst, psts[gl][:],
                            mybir.ActivationFunctionType.Identity,
                            bias=biast[:])
                # one output DMA per block, scalar ring (keeps the sync
                # ring free for input); the last block goes on sync (idle
                # by then) so its descgen overlaps scalar's previous block.
                ring = nc.sync if k == len(CHUNKS) - 1 else nc.scalar
                ring.dma_start(
                    out_dram.ap()[:, off * COLS:(off + ck) * COLS], stg[:])
                off += ck

    nc.compile()
    return nc


def _banded(weight: np.ndarray) -> np.ndarray:
    ball = np.zeros((128, KW * 128), dtype=np.float32)
    for dj in range(KW):
        for di in range(KH):
            m = np.arange(128 - di)
            ball[m + di, dj * 128 + m] = weight[di, dj]
    return ball


def kernel(x: np.ndarray, weight: np.ndarray, bias: np.ndarray) -> np.ndarray:
    global _compiled
    import ml_dtypes
    bf16 = ml_dtypes.bfloat16

    x = np.asarray(x, dtype=np.float32)
    weight = np.asarray(weight, dtype=np.float32)
    bias = np.asarray(bias, dtype=np.float32)

    if _compiled is None:
        _compiled = _build()
    nc = _compiled

    xpad = np.zeros((H, NCORES * COLS + KW - 1), dtype=bf16)
    xpad[:, :W] = x.astype(bf16)
    ball = _banded(weight).astype(bf16)
    bias_col = np.full((128, 1), bias[0], dtype=np.float32)

    # pack: xs = [banded weights | xp], xp[p, g*XC + c] = x[124g+p, 512c0+c]
    idx = (124 * np.arange(NG)[:, None] + np.arange(128)[None, :])  # (NG,128)
    in_maps = []
    for c in range(NCORES):
        xc = xpad[:, COLS * c: COLS * c + XC]      # (4096, XC) view
        xp = xc[idx, :]                            # (NG, 128, XC)
        xs = np.empty((128, BT + NG * XC), dtype=bf16)
        xs[:, :BT] = ball
        xs[:, BT:] = xp.transpose(1, 0, 2).reshape(128, NG * XC)
        in_maps.append({"xs": xs, "biast": bias_col})

    res = bass_utils.run_bass_kernel_spmd(nc, in_maps,
                                          core_ids=list(range(NCORES)),
                                          trace=TRACE)
    global LAST_EXEC_NS
    LAST_EXEC_NS = res.exec_time_ns

    # unpack: out[124g + m, 512c + n] = op[m, g*COLS + n]  (m < 124)
    cols = []
    for c in range(NCORES):
        op = res.results[c]["out"].reshape(128, NG, COLS)
        cols.append(op[:RV].transpose(1, 0, 2).reshape(OH, COLS))
    out = np.hstack(cols)
    return np.ascontiguousarray(out[:, :OW])


# revision 23
# speedup vs baseline: 1.0597x; 1.0187x over previous
"""Trainium2 Bass kernel: 4096x4096 valid 5x5 cross-correlation + scalar bias.

Strategy (8 NeuronCores, SPMD):
  - Shard the OUTPUT by columns: core c computes out[:, 512c : 512c+512]
    (core 7's last 4 columns are padding, trimmed after gather). Each core
    reads x rows 0..4095, cols [512c, 512c+516) (host-padded to width 4100).
  - On-core: the 5x5 conv is computed as banded-matrix matmuls on the
    TensorEngine. For an input row-tile X_g = x[124g : 124g+128, :] and
    kernel column dj, the banded matrix B_dj[k, m] = w[k-m, dj] gives
      (B_dj^T @ X_g[:, dj:dj+512])[m, n] = sum_di w[di, dj] x[124g+m+di, n+dj]
    so accumulating the 5 dj-matmuls in PSUM yields 124 valid output rows
    per tile. 4092 = 33 * 124 exactly; rows 124g+p for p<128, g<33 cover
    0..4095 exactly (124*32+127 = 4095).
  - HOST-PACKED I/O LAYOUT: x is pre-gathered on the host into a
    [128, 33*516] bf16 array whose partition p, segment g holds
    x[124g+p, :].  The whole input then streams in a handful of dma_starts
    with multi-KB contiguous lines per partition (descriptor- and
    semaphore-overhead amortized; the DMA issue path costs ~0.65us + 900ns
    semaphore per dma_start, so 100+ small DMAs dominated the old
    timeline).  The output is staged to SBUF as [128, 33*512] fp32 in the
    same packed form and written out in chunk-sized dma_starts; the host
    un-packs (transpose) after the gather.
  - PSUM accumulation is fp32; bias fused into the PSUM->SBUF drain (DVE
    tensor_scalar_add).  Matmul operands are bf16 (accumulate fp32), which
    halves input HBM traffic; rel err ~2e-3.
  - WARM-UP: the Tensor engine ramps 0.65->1.2->2.4 GHz over ~7us of
    sustained activity.  A run of dummy matmuls on a memset tile starts at
    t~0.5us (during the input DMA lead-in) so the real matmul stream runs
    at full clock from its first instruction.
"""
import os

os.environ.setdefault("MYCRO_LOCAL_CACHE", "1")

import numpy as np

import concourse.bass as bass
import concourse.bacc as bacc
import concourse.tile as tile
import concourse.mybir as mybir
from concourse import bass_utils

H, W = 4096, 4096
KH, KW = 5, 5
OH, OW = H - KH + 1, W - KW + 1          # 4092, 4092
NCORES = 8
COLS = 512                               # output cols per core
XC = COLS + KW - 1                       # 516 input cols per core
NG = 33                                  # row tiles per core (33*124 = 4092)
RV = 124                                 # valid output rows per tile

# groups per input-chunk == per output-block. First chunk small so the
# first matmul starts early; last chunks small so the final drain+DMA tail
# is short. PSUM: two adjacent blocks in flight need <= 8 banks.
CHUNKS = [1, 2, 4, 4, 4, 4, 4, 4, 3, 2, 1]
assert sum(CHUNKS) == NG and max(a + b for a, b in zip(CHUNKS, CHUNKS[1:])) <= 8

# Dummy matmuls bridge the idle window between Tensor-sequencer-ready
# (~7.3us, after the framework preamble) and first-chunk-consumable
# (~9us), keeping the clock ramp (0.65->1.2->2.4 GHz over ~4.5us of
# CONTINUOUS activity) going — an idle gap resets the ramp. 256-row warm
# matmuls (~0.2us each) give fine granularity so warm-up ends close to
# data-ready; real matmuls do useful work while the ramp completes.
WARM_MM = 12
WARM_ROWS = 256
BT = KW * 128                             # banded-weight cols at xs[:, 0:BT]

_compiled = None
TRACE = False            # test harness can flip this for neuron-profile timing
LAST_EXEC_NS = None


def _build():
    nc = bacc.Bacc("TRN2", target_bir_lowering=False, debug=False,
                   num_devices=NCORES)
    mdt = mybir.dt.bfloat16

    # xs = [banded weights | packed x]: one dtype, so the first dma_start
    # delivers the weights AND the first row-group together (each push
    # costs ~0.65us descgen + 0.9us semaphore on the critical path).
    x_dram = nc.dram_tensor("xs", (128, BT + NG * XC), mdt,
                            kind="ExternalInput")
    bias_dram = nc.dram_tensor("biast", (128, 1), mybir.dt.float32,
                               kind="ExternalInput")
    # NOTE: full 128 partitions on purpose — a 124-row DMA falls off the
    # HWDGE fast path (DIRECT2D descgen takes ~8us per push vs ~0.3us).
    out_dram = nc.dram_tensor("out", (128, NG * COLS), mybir.dt.float32,
                              kind="ExternalOutput")

    with tile.TileContext(nc) as tc:
        with (
            tc.tile_pool(name="const", bufs=1) as cpool,
            tc.tile_pool(name="x", bufs=len(CHUNKS)) as xpool,
            tc.tile_pool(name="stage", bufs=len(CHUNKS)) as spool,
            tc.tile_pool(name="psum", bufs=8, space=bass.MemorySpace.PSUM) as ppool,
        ):
            # PE warm-up: memset on GpSimd (idle at kernel start, so the
            # producer dependency resolves as early as possible), then
            # dummy matmuls into a scratch PSUM bank that is never read.
            warm = cpool.tile([128, COLS], mdt)
            nc.gpsimd.memset(warm[:], 0.0)
            wps = ppool.tile([128, COLS], mybir.dt.float32, name="warmps",
                             tag="ps")
            for i in range(WARM_MM):
                nc.tensor.matmul(wps[:, 0:WARM_ROWS], warm[:, 0:128],
                                 warm[:, 0:WARM_ROWS],
                                 start=True, stop=True)

            biast = cpool.tile([128, 1], mybir.dt.float32)
            bt = cpool.tile([128, BT], mdt)
            # Parallel critical-path loads: weights via sync ring, group 0
            # via the (otherwise idle until the first drain) scalar ring —
            # the two ~0.7us descgens and transfers overlap instead of
            # serializing, so the first real matmul starts ~1.5us earlier.
            nc.sync.dma_start(bt[:], x_dram.ap()[:, 0:BT])
            xts, off = [], 0
            for k, ck in enumerate(CHUNKS):
                xt = xpool.tile([128, ck * XC], mdt, tag="x")
                # chunk 0 rides the scalar ring: its descgen runs in
                # parallel with bt's on sync, so the first matmul's two
                # inputs arrive together despite the ~0.7us/push descgen
                # and ~1.4us post-transfer semaphore latency.
                ring = nc.scalar if k == 0 else nc.sync
                if k == 1:
                    # split: group 1 alone transfers ~0.4us sooner, meeting
                    # its consume deadline right after chunk 0's 5 matmuls
                    ring.dma_start(
                        xt[:, 0:XC],
                        x_dram.ap()[:, BT + off * XC:BT + (off + 1) * XC])
                    ring.dma_start(
                        xt[:, XC:2 * XC],
                        x_dram.ap()[:, BT + (off + 1) * XC:
                                    BT + (off + 2) * XC])
                else:
                    ring.dma_start(
                        xt[:],
                        x_dram.ap()[:, BT + off * XC:BT + (off + ck) * XC])
                xts.append(xt)
                off += ck
                if k == 0:
                    nc.scalar.dma_start(biast[:], bias_dram.ap())

            off = 0
            for k, ck in enumerate(CHUNKS):
                base = 0
                stg = spool.tile([128, ck * COLS], mybir.dt.float32)
                psts = []
                for gl in range(ck):
                    psts.append(ppool.tile([128, COLS], mybir.dt.float32,
                                           name=f"ps{off + gl}", tag="ps"))
                # weight-stationary sweep: dj outer, groups inner (pipelines
                # PSUM accumulation across banks)
                for dj in range(KW):
                    for gl in range(ck):
                        nc.tensor.matmul(
                            psts[gl][:],
                            bt[:, dj * 128:(dj + 1) * 128],
                            xts[k][:, base + gl * XC + dj:
                                   base + gl * XC + dj + COLS],
                            start=(dj == 0),
                            stop=(dj == KW - 1),
                        )
                last = k == len(CHUNKS) - 1
                if last:
                    # tail block (1 group): halve the critical chain by
                    # draining 256-col halves on DVE and ACT in parallel,
                    # then writing each half on its own ring
                    hw_ = COLS // 2
                    nc.vector.tensor_scalar_add(stg[:, 0:hw_],
                                                psts[0][:, 0:hw_], biast[:])
                    nc.scalar.activation(stg[:, hw_:COLS],
                                         psts[0][:, hw_:COLS],
                                         mybir.ActivationFunctionType.Identity,
                                         bias=biast[:])
                    nc.sync.dma_start(
                        out_dram.ap()[:, off * COLS:off * COLS + hw_],
                        stg[:, 0:hw_])
                    nc.scalar.dma_start(
                        out_dram.ap()[:, off * COLS + hw_:(off + 1) * COLS],
                        stg[:, hw_:COLS])
                    off += ck
                    continue
                # drain PSUM -> stage with fused bias, split across DVE and
                # ACT so a block's drains run in ~half the serial time
                for gl in range(ck):
                    dst = stg[:, gl * COLS:(gl + 1) * COLS]
                    if gl % 2 == 0:
                        nc.vector.tensor_scalar_add(dst, psts[gl][:],
                                                    biast[:])
                    else:
                        nc.scalar.activation(
                            dst, psts[gl][:],
                            mybir.ActivationFunctionType.Identity,
                            bias=biast[:])
                # one output DMA per block, scalar ring (keeps the sync
                # ring free for input)
                nc.scalar.dma_start(
                    out_dram.ap()[:, off * COLS:(off + ck) * COLS], stg[:])
                off += ck

    nc.compile()
    return nc


def _banded(weight: np.ndarray) -> np.ndarray:
    ball = np.zeros((128, KW * 128), dtype=np.float32)
    for dj in range(KW):
        for di in range(KH):
            m = np.arange(128 - di)
            ball[m + di, dj * 128 + m] = weight[di, dj]
    return ball


def kernel(x: np.ndarray, weight: np.ndarray, bias: np.ndarray) -> np.ndarray:
    global _compiled
    import ml_dtypes
    bf16 = ml_dtypes.bfloat16

    x = np.asarray(x, dtype=np.float32)
    weight = np.asarray(weight, dtype=np.float32)
    bias = np.asarray(bias, dtype=np.float32)

    if _compiled is None:
        _compiled = _build()
    nc = _compiled

    xpad = np.zeros((H, NCORES * COLS + KW - 1), dtype=bf16)
    xpad[:, :W] = x.astype(bf16)
    ball = _banded(weight).astype(bf16)
    bias_col = np.full((128, 1), bias[0], dtype=np.float32)

    # pack: xs = [banded weights | xp], xp[p, g*XC + c] = x[124g+p, 512c0+c]
    idx = (124 * np.arange(NG)[:, None] + np.arange(128)[None, :])  # (NG,128)
    in_maps = []
    for c in range(NCORES):
        xc = xpad[:, COLS * c: COLS * c + XC]      # (4096, XC) view
        xp = xc[idx, :]                            # (NG, 128, XC)
        xs = np.empty((128, BT + NG * XC), dtype=bf16)
        xs[:, :BT] = ball
        xs[:, BT:] = xp.transpose(1, 0, 2).reshape(128, NG * XC)
        in_maps.append({"xs": xs, "biast": bias_col})

    res = bass_utils.run_bass_kernel_spmd(nc, in_maps,
                                          core_ids=list(range(NCORES)),
                                          trace=TRACE)
    global LAST_EXEC_NS
    LAST_EXEC_NS = res.exec_time_ns

    # unpack: out[124g + m, 512c + n] = op[m, g*COLS + n]  (m < 124)
    cols = []
    for c in range(NCORES):
        op = res.results[c]["out"].reshape(128, NG, COLS)
        cols.append(op[:RV].transpose(1, 0, 2).reshape(OH, COLS))
    out = np.hstack(cols)
    return np.ascontiguousarray(out[:, :OW])
